# revision 32
# baseline (speedup 1.0000x reference)
"""GRU autoencoder Trainium2 kernel (bf16 col/row-tiled).

Data-parallel over batch: 8 cores x 64 rows. All gate matmuls are bf16 and
issued as column-tile pairs (tile_position (0,0)/(0,64)): each [128,512] PSUM
tile holds both 512-col halves of one gate (top partitions = cols 0-511,
bottom = cols 512-1023), so two M=64 matmuls run concurrently on the PE
array (~2x). Activations then run full-lane on [128,512] tiles, and their
stacked layout feeds row-tile-paired single-pass bf16 transposes
(tile_position (0,0)/(64,0)) that put z/n back into the transposed hidden
layout for the h-update. Bias seeding is a col-paired K=128 matmul against
row-replicated bias tiles. Decoder z-outputs pair two timesteps per column
duo. Hidden state is carried as bf16 in transposed layout
(hT[klo, 64*khi+b] = h[b, 128*khi+klo]).
"""
import os
import sys
import types

import ml_dtypes
import numpy as np

import concourse.bass as bass
import concourse.mybir as mybir
import concourse.tile as tile
from concourse import bass_utils

F32 = mybir.dt.float32
BF16 = mybir.dt.bfloat16
FP16 = mybir.dt.float16
AF = mybir.ActivationFunctionType
OP = mybir.AluOpType

N_CORES = 8
B, T, I, H = 512, 128, 512, 1024
BL = B // N_CORES  # 64


# ---------------------------------------------------------------- fixups
def _split_multi_waits(nc, max_waits=1):
    """This walrus build allows only one sync-wait per instruction; hoist
    excess waits onto preceding NoOps (same engine, so semantics hold)."""
    for f in nc.m.functions:
        for blk in f.blocks:
            insts = blk.instructions
            if not any(
                i.sync_info is not None
                and i.sync_info.on_wait
                and len(i.sync_info.on_wait) > max_waits
                for i in insts
            ):
                continue
            new = []
            for inst in insts:
                si = inst.sync_info
                if si is not None and si.on_wait and len(si.on_wait) > max_waits:
                    waits = list(si.on_wait)
                    extra, keep = waits[:-max_waits], waits[-max_waits:]
                    for cs in range(0, len(extra), max_waits):
                        nop = mybir.InstNoOp(
                            name=nc.get_next_instruction_name(),
                            engine=inst.engine,
                            ins=[],
                            outs=[],
                            sync_info=mybir.SyncInfo(
                                on_wait=extra[cs : cs + max_waits], on_update=[]
                            ),
                        )
                        nc.register_instruction(nop)
                        new.append(nop)
                    si.on_wait = keep
                new.append(inst)
            insts[:] = new


def _install_ntff_hook():
    if "antenv.axon_hooks" in sys.modules:
        return True
    mod = types.ModuleType("antenv.axon_hooks")
    state = {"hook": None}
    mod.set_axon_ntff_profile_hook = lambda h: state.__setitem__("hook", h)
    mod.get_axon_ntff_profile_hook = lambda: state["hook"]
    sys.modules["antenv.axon_hooks"] = mod
    try:
        import antenv

        antenv.axon_hooks = mod
        from trn_agent_boot.trn_boot import _ntff_profile_via_ctypes

        hook = _ntff_profile_via_ctypes("/opt/axon/libaxon_pjrt.so")
        if hook is None:
            return False
        mod.set_axon_ntff_profile_hook(hook)
        return True
    except Exception:
        return False


# ---------------------------------------------------------------- program
def build_nc(n_steps=T):
    nc = bass.Bass("TRN2", target_bir_lowering=False, debug=False, num_devices=N_CORES)

    xT_d = nc.dram_tensor("xT", [n_steps, 4, 128, BL], BF16, kind="ExternalInput").ap()
    wih_d = nc.dram_tensor("wihT", [4, 128, 3 * H], BF16, kind="ExternalInput").ap()
    whh_d = nc.dram_tensor("whhT", [8, 128, 3 * H], BF16, kind="ExternalInput").ap()
    wcb_d = nc.dram_tensor("wcombT", [8, 128, 4 * H], BF16, kind="ExternalInput").ap()
    wz_d = nc.dram_tensor("wzT", [8, 128, I], BF16, kind="ExternalInput").ap()
    be_d = nc.dram_tensor("bias_enc", [128, 4 * H], BF16, kind="ExternalInput").ap()
    bd_d = nc.dram_tensor("bias_dec", [128, 4 * H], BF16, kind="ExternalInput").ap()
    bz_d = nc.dram_tensor("bz_rep", [128, I], BF16, kind="ExternalInput").ap()
    ss_d = nc.dram_tensor("sstat", [128, 64], BF16, kind="ExternalInput").ap()
    id_d = nc.dram_tensor("iden2", [128, 64], BF16, kind="ExternalInput").ap()
    ih_d = nc.dram_tensor("iden16", [128, 128], FP16, kind="ExternalInput").ap()
    h0_d = nc.dram_tensor("h0T", [128, 512], BF16, kind="ExternalInput").ap()
    z_d = nc.dram_tensor("z", [BL, n_steps, I], F32, kind="ExternalOutput").ap()

    # gate -> column offset in the 3H/4H weight layout (PyTorch order r,z,n)
    ENC_C0 = {"r": 0, "z": H, "in": 2 * H}
    DEC_C0 = {"r": 0, "z": H, "in": 2 * H, "hn": 3 * H}
    BIAS_C0 = {"r": 0, "z": H, "in": 2 * H, "hn": 3 * H}

    with tile.TileContext(nc) as tc:
        with (
            tc.tile_pool(name="wgt", bufs=1) as wgt,
            tc.tile_pool(name="cst", bufs=1) as cst,
            tc.tile_pool(name="hst", bufs=5) as hst,
            tc.tile_pool(name="xts", bufs=4) as xts,
            tc.tile_pool(name="gsb", bufs=2) as gsb,
            tc.tile_pool(name="tmp", bufs=2) as tmpp,
            tc.tile_pool(name="zo", bufs=2) as zop,
            tc.tile_pool(name="ps", bufs=8, space="PSUM") as ps,
        ):
            sstat = cst.tile([128, 64], BF16)
            nc.sync.dma_start(sstat[:], ss_d[:])
            iden2 = cst.tile([128, 64], BF16)
            nc.sync.dma_start(iden2[:], id_d[:])
            iden16 = cst.tile([128, 128], FP16)
            nc.sync.dma_start(iden16[:], ih_d[:])
            bias_enc = cst.tile([128, 4 * H], BF16)
            nc.sync.dma_start(bias_enc[:], be_d[:])
            bias_dec = cst.tile([128, 4 * H], BF16)
            nc.scalar.dma_start(bias_dec[:], bd_d[:])
            bz_rep = cst.tile([128, I], BF16)
            nc.sync.dma_start(bz_rep[:], bz_d[:])
            hT = hst.tile([128, 512], BF16, tag="h")
            nc.sync.dma_start(hT[:], h0_d[:])

            # encoder-critical DMAs first (x prologue between wih and whh so
            # the first steps' inputs and weights all arrive early); decoder
            # weights follow and overlap with encoder compute
            xt_tiles = {}
            wih = wgt.tile([128, 4, 3 * H], BF16)
            for k in range(4):
                nc.sync.dma_start(wih[:, k, :], wih_d[k])
            for t in range(min(3, n_steps)):
                xt_tiles[t] = xts.tile([128, 4, BL], BF16, tag="x", name=f"xt{t}")
                for k in range(4):
                    nc.sync.dma_start(xt_tiles[t][:, k, :], xT_d[t, k])
            whh = wgt.tile([128, 8, 3 * H], BF16)
            for k in range(8):
                nc.sync.dma_start(whh[:, k, :], whh_d[k])
            wcb = wgt.tile([128, 8, 4 * H], BF16)
            for k in range(8):
                nc.scalar.dma_start(wcb[:, k, :], wcb_d[k])
            wz = wgt.tile([128, 8, I], BF16)
            for k in range(8):
                nc.scalar.dma_start(wz[:, k, :], wz_d[k])

            def dup(t, stat_lo, mov_lo, stat_hi, mov_hi, start, stop):
                """One column-tile duo: two concurrent M=64 matmuls."""
                nc.tensor.matmul(t[0:64, :], stat_lo, mov_lo, start=start,
                                 stop=stop, tile_position=(0, 0))
                nc.tensor.matmul(t[64:128, :], stat_hi, mov_hi, start=start,
                                 stop=stop, tile_position=(0, 64))

            def seed_tile(nm, t_id, bias, gate):
                t = ps.tile([128, 512], F32, tag="ps", name=f"{nm}{t_id}")
                c0 = BIAS_C0[gate]
                dup(t, sstat[:, :], bias[:, c0 : c0 + 512],
                    sstat[:, :], bias[:, c0 + 512 : c0 + 1024],
                    start=True, stop=False)
                return t

            def emit_gi(g, xt, gates, ks=range(4)):
                for gate in gates:
                    c0 = ENC_C0[gate]
                    t = g[gate]
                    for k in ks:
                        dup(t, xt[:, k, :], wih[:, k, c0 : c0 + 512],
                            xt[:, k, :], wih[:, k, c0 + 512 : c0 + 1024],
                            start=False, stop=False)

            def emit_gh(g, w, c0map, gates, no_stop=()):
                for gate in gates:
                    c0 = c0map[gate]
                    t = g[gate]
                    for k in range(8):
                        hs = hT[:, 64 * k : 64 * k + 64]
                        dup(t, hs, w[:, k, c0 : c0 + 512],
                            hs, w[:, k, c0 + 512 : c0 + 1024],
                            start=False, stop=(k == 7 and gate not in no_stop))

            def transpose_into(src_sb, pT0, pT8, c0):
                """Row-tile-paired transpose of a stacked [128,512] source into
                pT0 (h-cols 0-511 -> cols c0..c0+256) and pT8 (512-1023)."""
                for j in range(4):
                    nc.tensor.matmul(
                        pT0[:, c0 + 64 * j : c0 + 64 * j + 64],
                        src_sb[0:64, 128 * j : 128 * j + 128],
                        iden2[0:64, :], start=True, stop=True,
                        tile_position=(0, 0),
                    )
                    nc.tensor.matmul(
                        pT8[:, c0 + 64 * j : c0 + 64 * j + 64],
                        src_sb[64:128, 128 * j : 128 * j + 128],
                        iden2[64:128, :], start=True, stop=True,
                        tile_position=(64, 0),
                    )

            def step_tail(t_id, g, fill_a, fills=()):
                """sigmoids, z-transposes + early A/z_bar, n-chain with PE
                inject, paired n-transposes, quarter-wise h-update.
                fill_a + fills[0..2]: independent PE duos emitted at the
                points where the in-order PE queue would otherwise stall
                (after inject / between n-transpose pairs / before update)."""
                nonlocal hT

                def _f(i):
                    if i < len(fills) and fills[i] is not None:
                        fills[i]()

                z_sb = gsb.tile([128, 512], BF16, tag="z", name=f"z{t_id}")
                nc.scalar.activation(z_sb[:], g["z"][:], AF.Sigmoid)
                r_sb = gsb.tile([128, 512], BF16, tag="r", name=f"r{t_id}")
                nc.scalar.activation(r_sb[:], g["r"][:], AF.Sigmoid)

                # transpose z; zbT = 1 - zT and A = zT*hT (DVE, off-critical)
                pT0 = ps.tile([128, 512], F32, tag="ps", name=f"pT0_{t_id}")
                pT8 = ps.tile([128, 512], F32, tag="ps", name=f"pT8_{t_id}")
                transpose_into(z_sb, pT0, pT8, 0)
                zb = tmpp.tile([128, 512], BF16, tag="zb", name=f"zb{t_id}")
                a_sb = tmpp.tile([128, 512], F32, tag="a", name=f"a{t_id}")
                for hh, pT in ((0, pT0), (1, pT8)):
                    s = slice(256 * hh, 256 * hh + 256)
                    nc.vector.tensor_scalar(zb[:, s], pT[:, 0:256], -1.0, 1.0,
                                            OP.mult, OP.add)
                    nc.vector.tensor_mul(a_sb[:, s], pT[:, 0:256], hT[:, s])

                if fill_a is not None:
                    fill_a()

                # n = tanh(in + r*hn): r*hn on DVE (fp16), injected into the
                # `in` PSUM group via identity matmul duos; tanh + n-transpose
                # pipelined by column halves with filler between PE stages
                rhn = tmpp.tile([128, 512], FP16, tag="rhn", name=f"rhn{t_id}")
                n_sb = gsb.tile([128, 512], BF16, tag="n", name=f"n{t_id}")
                for cc in (0, 1):
                    s = slice(256 * cc, 256 * cc + 256)
                    nc.vector.tensor_mul(rhn[:, s], r_sb[:, s], g["hn"][:, s])
                    nc.tensor.matmul(g["in"][0:64, s], iden16[:, 0:64],
                                     rhn[:, s], start=False, stop=True,
                                     tile_position=(0, 0))
                    nc.tensor.matmul(g["in"][64:128, s], iden16[:, 64:128],
                                     rhn[:, s], start=False, stop=True,
                                     tile_position=(0, 64))

                _f(0)

                for cc in (0, 1):
                    s = slice(256 * cc, 256 * cc + 256)
                    nc.scalar.activation(n_sb[:, s], g["in"][:, s], AF.Tanh)
                    for j in (2 * cc, 2 * cc + 1):
                        nc.tensor.matmul(
                            pT0[:, 256 + 64 * j : 256 + 64 * j + 64],
                            n_sb[0:64, 128 * j : 128 * j + 128],
                            iden2[0:64, :], start=True, stop=True,
                            tile_position=(0, 0))
                        nc.tensor.matmul(
                            pT8[:, 256 + 64 * j : 256 + 64 * j + 64],
                            n_sb[64:128, 128 * j : 128 * j + 128],
                            iden2[64:128, :], start=True, stop=True,
                            tile_position=(64, 0))
                    if cc == 0:
                        _f(1)

                _f(2)

                # hT' = A + zbT*nT by [128,128] quarters, low-k first so the
                # next step's low-k gh duos release early
                hT_new = hst.tile([128, 512], BF16, tag="h", name=f"h{t_id}")
                for q, pT in ((0, pT0), (1, pT0), (2, pT8), (3, pT8)):
                    s = slice(128 * q, 128 * q + 128)
                    ns = slice(256 + 128 * (q % 2), 256 + 128 * (q % 2) + 128)
                    d = tmpp.tile([128, 128], F32, tag="d", name=f"d{t_id}_{q}")
                    nc.vector.tensor_mul(d[:], zb[:, s], pT[:, ns])
                    nc.vector.tensor_add(hT_new[:, s], a_sb[:, s], d[:])
                hT = hT_new

            # ================= encoder =================
            def seed_and_gi_zr(t_id, xt):
                g = {}
                g["z"] = seed_tile("pz", t_id, bias_enc, "z")
                g["r"] = seed_tile("pr", t_id, bias_enc, "r")
                emit_gi(g, xt, ("z",))
                emit_gi(g, xt, ("r",), ks=(0, 1))
                return g

            cur = seed_and_gi_zr(0, xt_tiles[0])
            emit_gi(cur, xt_tiles[0], ("r",), ks=(2, 3))
            cur["in"] = seed_tile("pi", 0, bias_enc, "in")
            emit_gi(cur, xt_tiles[0], ("in",))
            cur["hn"] = seed_tile("ph", 0, bias_enc, "hn")

            for t in range(n_steps):
                if t + 3 < n_steps:
                    xt_tiles[t + 3] = xts.tile([128, 4, BL], BF16, tag="x",
                                               name=f"xt{t+3}")
                    for k in range(4):
                        nc.sync.dma_start(xt_tiles[t + 3][:, k, :], xT_d[t + 3, k])
                emit_gh(cur, whh, ENC_C0, ("z", "r"))
                emit_gh(cur, whh, {"hn": 2 * H}, ("hn",))
                g = cur
                nxt = {}
                if t + 1 < n_steps:
                    xt_next = xt_tiles[t + 1]

                    def fill_a(nxt=nxt, xt_next=xt_next, t=t):
                        nxt.update(seed_and_gi_zr(t + 1, xt_next))

                    def fb1(nxt=nxt, xt_next=xt_next, t=t):
                        nxt["in"] = seed_tile("pi", t + 1, bias_enc, "in")
                        emit_gi(nxt, xt_next, ("in",), ks=(0, 1))

                    def fb2(nxt=nxt, xt_next=xt_next):
                        emit_gi(nxt, xt_next, ("in",), ks=(2, 3))

                    def fb3(nxt=nxt, xt_next=xt_next, t=t):
                        emit_gi(nxt, xt_next, ("r",), ks=(2, 3))
                        nxt["hn"] = seed_tile("ph", t + 1, bias_enc, "hn")

                    fills = (fb1, fb2, fb3)
                else:
                    fill_a, fills = None, ()
                step_tail(t, g, fill_a, fills)
                cur = nxt
                xt_tiles.pop(t, None)

            # ================= decoder =================
            def dec_seeds(g, t_id, gates):
                for gate in gates:
                    g[gate] = seed_tile(f"d{gate}", t_id, bias_dec, gate)
                return g

            def emit_zfill(hA, hB, o0, o1, ks):
                """Column-duo z-output for two steps: top half <- hA (out o0),
                bottom <- hB (out o1), k-chunks `ks` of the accumulation."""
                t = zfill_ps[0]
                for k in ks:
                    dup(t, hA[:, 64 * k : 64 * k + 64], wz[:, k, :],
                        hB[:, 64 * k : 64 * k + 64], wz[:, k, :],
                        start=False, stop=(k == 7))
                if ks and ks[-1] == 7:
                    zo_sb = zop.tile([128, 512], F32, tag="zo", name=f"zo{o1}")
                    nc.scalar.copy(zo_sb[:], t[:])
                    nc.sync.dma_start(z_d[:, o0, :], zo_sb[0:64, :])
                    nc.sync.dma_start(z_d[:, o1, :], zo_sb[64:128, :])

            def zfill_seed(o1):
                t = ps.tile([128, 512], F32, tag="ps", name=f"pzo{o1}")
                dup(t, sstat[:, :], bz_rep[:, :], sstat[:, :], bz_rep[:, :],
                    start=True, stop=False)
                zfill_ps[0] = t

            # z-output pair (o, o+1) reads entry(o+1)=H_o and entry(o+2)=
            # H_{o+1}; its 9 duos are spread over steps o+3 (seed + k0-3) and
            # o+4 (k4-7 + copy) so every decoder step gets ~4.5 filler duos
            assert n_steps == 1 or n_steps % 2 == 0
            zfill_ps = [None]
            entries = {}
            cur = dec_seeds({}, 1000, ("z", "r", "in", "hn"))
            for t in range(n_steps):
                entries[t] = hT
                emit_gh(cur, wcb, DEC_C0, ("z", "r"))
                emit_gh(cur, wcb, DEC_C0, ("hn",))
                emit_gh(cur, wcb, DEC_C0, ("in",), no_stop=("in",))
                g = cur
                nxt = {}
                last = t + 1 >= n_steps

                def fill_a(nxt=nxt, t=t, last=last):
                    if not last:
                        dec_seeds(nxt, 1001 + t, ("z", "r"))
                    if t % 2 == 1 and t >= 3:
                        zfill_seed(t - 3)
                        emit_zfill(entries[t - 2], entries[t - 1],
                                   t - 3, t - 2, (0,))
                    elif t % 2 == 0 and t >= 4:
                        emit_zfill(entries[t - 3], entries[t - 2],
                                   t - 4, t - 3, (4,))

                def fb1(nxt=nxt, t=t, last=last):
                    if not last:
                        dec_seeds(nxt, 1001 + t, ("in", "hn"))

                def fb2(t=t):
                    if t % 2 == 1 and t >= 3:
                        emit_zfill(entries[t - 2], entries[t - 1],
                                   t - 3, t - 2, (1,))
                    elif t % 2 == 0 and t >= 4:
                        emit_zfill(entries[t - 3], entries[t - 2],
                                   t - 4, t - 3, (5,))

                def fb3(t=t):
                    if t % 2 == 1 and t >= 3:
                        emit_zfill(entries[t - 2], entries[t - 1],
                                   t - 3, t - 2, (2, 3))
                    elif t % 2 == 0 and t >= 4:
                        emit_zfill(entries[t - 3], entries[t - 2],
                                   t - 4, t - 3, (6, 7))

                step_tail(1000 + t, g, fill_a, (fb1, fb2, fb3))
                cur = nxt

            if n_steps >= 4:
                emit_zfill(entries[n_steps - 3], entries[n_steps - 2],
                           n_steps - 4, n_steps - 3, list(range(4, 8)))
            if n_steps >= 2:
                zfill_seed(n_steps - 1)
                emit_zfill(entries[n_steps - 1], hT, n_steps - 2, n_steps - 1,
                           list(range(8)))
            elif n_steps == 1:
                zfill_seed(0)
                emit_zfill(hT, hT, 0, 0, list(range(8)))
    return nc


# ---------------------------------------------------------------- host side
def _prep_shared(enc_Wih, enc_Whh, enc_bih, enc_bhh,
                 dec_Wih, dec_Whh, dec_bih, dec_bhh, Wz, bz):
    bf = ml_dtypes.bfloat16
    f32 = np.float32

    def tobf(a):
        return np.ascontiguousarray(np.asarray(a, f32)).astype(bf)

    wihT = tobf(enc_Wih.T.reshape(I, 3 * H)).reshape(4, 128, 3 * H)
    whhT = tobf(enc_Whh.T).reshape(8, 128, 3 * H)
    wcomb = np.concatenate(
        [dec_Wih[: 2 * H] + dec_Whh[: 2 * H], dec_Wih[2 * H :], dec_Whh[2 * H :]], 0
    )
    wcombT = tobf(wcomb.T).reshape(8, 128, 4 * H)
    wzT = tobf(np.asarray(Wz, f32).T).reshape(8, 128, I)

    def rep(row):
        return np.broadcast_to(np.asarray(row, f32)[None, :], (128, row.shape[0]))

    be = np.concatenate([np.asarray(enc_bih, f32)[: 2 * H]
                         + np.asarray(enc_bhh, f32)[: 2 * H],
                         np.asarray(enc_bih, f32)[2 * H :],
                         np.asarray(enc_bhh, f32)[2 * H :]])
    bd = np.concatenate([np.asarray(dec_bih, f32)[: 2 * H]
                         + np.asarray(dec_bhh, f32)[: 2 * H],
                         np.asarray(dec_bih, f32)[2 * H :],
                         np.asarray(dec_bhh, f32)[2 * H :]])
    iden2 = np.concatenate([np.eye(64, dtype=f32)] * 2, axis=0)
    iden16 = np.zeros((128, 128), f32)
    iden16[0:64, 0:64] = np.eye(64, dtype=f32)
    iden16[64:128, 64:128] = np.eye(64, dtype=f32)
    return {
        "wihT": wihT, "whhT": whhT, "wcombT": wcombT, "wzT": wzT,
        "bias_enc": tobf(rep(be)), "bias_dec": tobf(rep(bd)),
        "bz_rep": tobf(rep(np.asarray(bz, f32))),
        "sstat": np.full((128, 64), 1.0 / 128, f32).astype(bf),
        "iden2": tobf(iden2),
        "iden16": iden16.astype(np.float16),
        "h0T": np.full((128, 512), 0.1, f32).astype(bf),
    }


def kernel(x, enc_Wih, enc_Whh, enc_bih, enc_bhh,
           dec_Wih, dec_Whh, dec_bih, dec_bhh, Wz, bz, n_steps=T):
    x = np.asarray(x, np.float32)
    shared = _prep_shared(enc_Wih, enc_Whh, enc_bih, enc_bhh,
                          dec_Wih, dec_Whh, dec_bih, dec_bhh, Wz, bz)
    in_maps = []
    for c in range(N_CORES):
        xc = x[c * BL : (c + 1) * BL, :n_steps]  # [BL, n_steps, I]
        xT = np.ascontiguousarray(xc.transpose(1, 2, 0)).reshape(n_steps, 4, 128, BL)
        in_maps.append({"xT": xT.astype(ml_dtypes.bfloat16), **shared})

    nc = build_nc(n_steps)
    _split_multi_waits(nc)

    trace = bool(int(os.environ.get("GRU_TRACE", "0")))
    if trace:
        _install_ntff_hook()
    res = bass_utils.run_bass_kernel_spmd(
        nc, in_maps, core_ids=list(range(N_CORES)), trace=trace
    )
    if trace and res.exec_time_ns is not None:
        print(f"HW exec time: {res.exec_time_ns} ns")
    out = np.concatenate([res.results[c]["z"] for c in range(N_CORES)], axis=0)
    return out


# revision 36
# speedup vs baseline: 1.0071x; 1.0071x over previous
"""GRU autoencoder Trainium2 kernel (bf16 col/row-tiled).

Data-parallel over batch: 8 cores x 64 rows. All gate matmuls are bf16 and
issued as column-tile pairs (tile_position (0,0)/(0,64)): each [128,512] PSUM
tile holds both 512-col halves of one gate (top partitions = cols 0-511,
bottom = cols 512-1023), so two M=64 matmuls run concurrently on the PE
array (~2x). Activations then run full-lane on [128,512] tiles, and their
stacked layout feeds row-tile-paired single-pass bf16 transposes
(tile_position (0,0)/(64,0)) that put z/n back into the transposed hidden
layout for the h-update. Bias seeding is a col-paired K=128 matmul against
row-replicated bias tiles. Decoder z-outputs pair two timesteps per column
duo. Hidden state is carried as bf16 in transposed layout
(hT[klo, 64*khi+b] = h[b, 128*khi+klo]).
"""
import os
import sys
import types

import ml_dtypes
import numpy as np

import concourse.bass as bass
import concourse.mybir as mybir
import concourse.tile as tile
from concourse import bass_utils

F32 = mybir.dt.float32
BF16 = mybir.dt.bfloat16
FP16 = mybir.dt.float16
AF = mybir.ActivationFunctionType
OP = mybir.AluOpType

N_CORES = 8
B, T, I, H = 512, 128, 512, 1024
BL = B // N_CORES  # 64


# ---------------------------------------------------------------- fixups
def _split_multi_waits(nc, max_waits=1):
    """This walrus build allows only one sync-wait per instruction; hoist
    excess waits onto preceding NoOps (same engine, so semantics hold)."""
    for f in nc.m.functions:
        for blk in f.blocks:
            insts = blk.instructions
            if not any(
                i.sync_info is not None
                and i.sync_info.on_wait
                and len(i.sync_info.on_wait) > max_waits
                for i in insts
            ):
                continue
            new = []
            for inst in insts:
                si = inst.sync_info
                if si is not None and si.on_wait and len(si.on_wait) > max_waits:
                    waits = list(si.on_wait)
                    extra, keep = waits[:-max_waits], waits[-max_waits:]
                    for cs in range(0, len(extra), max_waits):
                        nop = mybir.InstNoOp(
                            name=nc.get_next_instruction_name(),
                            engine=inst.engine,
                            ins=[],
                            outs=[],
                            sync_info=mybir.SyncInfo(
                                on_wait=extra[cs : cs + max_waits], on_update=[]
                            ),
                        )
                        nc.register_instruction(nop)
                        new.append(nop)
                    si.on_wait = keep
                new.append(inst)
            insts[:] = new


def _install_ntff_hook():
    if "antenv.axon_hooks" in sys.modules:
        return True
    mod = types.ModuleType("antenv.axon_hooks")
    state = {"hook": None}
    mod.set_axon_ntff_profile_hook = lambda h: state.__setitem__("hook", h)
    mod.get_axon_ntff_profile_hook = lambda: state["hook"]
    sys.modules["antenv.axon_hooks"] = mod
    try:
        import antenv

        antenv.axon_hooks = mod
        from trn_agent_boot.trn_boot import _ntff_profile_via_ctypes

        hook = _ntff_profile_via_ctypes("/opt/axon/libaxon_pjrt.so")
        if hook is None:
            return False
        mod.set_axon_ntff_profile_hook(hook)
        return True
    except Exception:
        return False


# ---------------------------------------------------------------- program
def build_nc(n_steps=T):
    nc = bass.Bass("TRN2", target_bir_lowering=False, debug=False, num_devices=N_CORES)

    xT_d = nc.dram_tensor("xT", [n_steps, 4, 128, BL], BF16, kind="ExternalInput").ap()
    wih_d = nc.dram_tensor("wihT", [4, 128, 3 * H], BF16, kind="ExternalInput").ap()
    whh_d = nc.dram_tensor("whhT", [8, 128, 3 * H], BF16, kind="ExternalInput").ap()
    wcb_d = nc.dram_tensor("wcombT", [8, 128, 4 * H], BF16, kind="ExternalInput").ap()
    wz_d = nc.dram_tensor("wzT", [8, 128, I], BF16, kind="ExternalInput").ap()
    be_d = nc.dram_tensor("bias_enc", [128, 4 * H], BF16, kind="ExternalInput").ap()
    bd_d = nc.dram_tensor("bias_dec", [128, 4 * H], BF16, kind="ExternalInput").ap()
    bz_d = nc.dram_tensor("bz_rep", [128, I], BF16, kind="ExternalInput").ap()
    ss_d = nc.dram_tensor("sstat", [128, 64], BF16, kind="ExternalInput").ap()
    id_d = nc.dram_tensor("iden2", [128, 64], BF16, kind="ExternalInput").ap()
    ih_d = nc.dram_tensor("iden16", [128, 128], FP16, kind="ExternalInput").ap()
    h0_d = nc.dram_tensor("h0T", [128, 512], BF16, kind="ExternalInput").ap()
    z_d = nc.dram_tensor("z", [BL, n_steps, I], F32, kind="ExternalOutput").ap()

    # gate -> column offset in the 3H/4H weight layout (PyTorch order r,z,n)
    ENC_C0 = {"r": 0, "z": H, "in": 2 * H}
    DEC_C0 = {"r": 0, "z": H, "in": 2 * H, "hn": 3 * H}
    BIAS_C0 = {"r": 0, "z": H, "in": 2 * H, "hn": 3 * H}

    with tile.TileContext(nc) as tc:
        with (
            tc.tile_pool(name="wgt", bufs=1) as wgt,
            tc.tile_pool(name="cst", bufs=1) as cst,
            tc.tile_pool(name="hst", bufs=5) as hst,
            tc.tile_pool(name="xts", bufs=4) as xts,
            tc.tile_pool(name="gsb", bufs=2) as gsb,
            tc.tile_pool(name="tmp", bufs=2) as tmpp,
            tc.tile_pool(name="zo", bufs=2) as zop,
            tc.tile_pool(name="ps", bufs=8, space="PSUM") as ps,
        ):
            sstat = cst.tile([128, 64], BF16)
            nc.sync.dma_start(sstat[:], ss_d[:])
            iden2 = cst.tile([128, 64], BF16)
            nc.sync.dma_start(iden2[:], id_d[:])
            iden16 = cst.tile([128, 128], FP16)
            nc.sync.dma_start(iden16[:], ih_d[:])
            bias_enc = cst.tile([128, 4 * H], BF16)
            nc.sync.dma_start(bias_enc[:], be_d[:])
            bias_dec = cst.tile([128, 4 * H], BF16)
            bz_rep = cst.tile([128, I], BF16)
            nc.sync.dma_start(bz_rep[:], bz_d[:])
            hT = hst.tile([128, 512], BF16, tag="h")
            nc.sync.dma_start(hT[:], h0_d[:])

            # encoder-critical DMAs first (x prologue between wih and whh so
            # the first steps' inputs and weights all arrive early); decoder
            # weights follow and overlap with encoder compute
            xt_tiles = {}
            wih = wgt.tile([128, 4, 3 * H], BF16)
            for k in range(4):
                nc.sync.dma_start(wih[:, k, :], wih_d[k])
            for t in range(min(3, n_steps)):
                xt_tiles[t] = xts.tile([128, 4, BL], BF16, tag="x", name=f"xt{t}")
                for k in range(4):
                    nc.sync.dma_start(xt_tiles[t][:, k, :], xT_d[t, k])
            # whh chunks split across both DGE queues so the encoder's first
            # gh duos aren't starved behind a single-queue weight load
            whh = wgt.tile([128, 8, 3 * H], BF16)
            for k in range(8):
                eng = nc.sync if k % 2 == 0 else nc.scalar
                eng.dma_start(whh[:, k, :], whh_d[k])
            nc.scalar.dma_start(bias_dec[:], bd_d[:])
            wcb = wgt.tile([128, 8, 4 * H], BF16)
            for k in range(8):
                nc.scalar.dma_start(wcb[:, k, :], wcb_d[k])
            wz = wgt.tile([128, 8, I], BF16)
            for k in range(8):
                nc.scalar.dma_start(wz[:, k, :], wz_d[k])

            def dup(t, stat_lo, mov_lo, stat_hi, mov_hi, start, stop):
                """One column-tile duo: two concurrent M=64 matmuls."""
                nc.tensor.matmul(t[0:64, :], stat_lo, mov_lo, start=start,
                                 stop=stop, tile_position=(0, 0))
                nc.tensor.matmul(t[64:128, :], stat_hi, mov_hi, start=start,
                                 stop=stop, tile_position=(0, 64))

            def seed_tile(nm, t_id, bias, gate):
                t = ps.tile([128, 512], F32, tag="ps", name=f"{nm}{t_id}")
                c0 = BIAS_C0[gate]
                dup(t, sstat[:, :], bias[:, c0 : c0 + 512],
                    sstat[:, :], bias[:, c0 + 512 : c0 + 1024],
                    start=True, stop=False)
                return t

            def emit_gi(g, xt, gates, ks=range(4)):
                for gate in gates:
                    c0 = ENC_C0[gate]
                    t = g[gate]
                    for k in ks:
                        dup(t, xt[:, k, :], wih[:, k, c0 : c0 + 512],
                            xt[:, k, :], wih[:, k, c0 + 512 : c0 + 1024],
                            start=False, stop=False)

            # k-order matches the h-update quarter completion order
            # (khi01, khi45, khi23, khi67)
            GH_KS = (0, 1, 4, 5, 2, 3, 6, 7)

            def emit_gh(g, w, c0map, gates, no_stop=()):
                for gate in gates:
                    c0 = c0map[gate]
                    t = g[gate]
                    for k in GH_KS:
                        hs = hT[:, 64 * k : 64 * k + 64]
                        dup(t, hs, w[:, k, c0 : c0 + 512],
                            hs, w[:, k, c0 + 512 : c0 + 1024],
                            start=False, stop=(k == 7 and gate not in no_stop))

            def transpose_into(src_sb, pT0, pT8, c0):
                """Row-tile-paired transpose of a stacked [128,512] source into
                pT0 (h-cols 0-511 -> cols c0..c0+256) and pT8 (512-1023)."""
                for j in range(4):
                    nc.tensor.matmul(
                        pT0[:, c0 + 64 * j : c0 + 64 * j + 64],
                        src_sb[0:64, 128 * j : 128 * j + 128],
                        iden2[0:64, :], start=True, stop=True,
                        tile_position=(0, 0),
                    )
                    nc.tensor.matmul(
                        pT8[:, c0 + 64 * j : c0 + 64 * j + 64],
                        src_sb[64:128, 128 * j : 128 * j + 128],
                        iden2[64:128, :], start=True, stop=True,
                        tile_position=(64, 0),
                    )

            def step_tail(t_id, g, fill_a, fills=()):
                """sigmoids, z-transposes + early A/z_bar, n-chain with PE
                inject, paired n-transposes, quarter-wise h-update.
                fill_a + fills[0..2]: independent PE duos emitted at the
                points where the in-order PE queue would otherwise stall
                (after inject / between n-transpose pairs / before update)."""
                nonlocal hT

                def _f(i):
                    if i < len(fills) and fills[i] is not None:
                        fills[i]()

                z_sb = gsb.tile([128, 512], BF16, tag="z", name=f"z{t_id}")
                nc.scalar.activation(z_sb[:], g["z"][:], AF.Sigmoid)
                r_sb = gsb.tile([128, 512], BF16, tag="r", name=f"r{t_id}")
                nc.scalar.activation(r_sb[:], g["r"][:], AF.Sigmoid)

                # transpose z; zbT = 1 - zT and A = zT*hT (DVE, off-critical)
                pT0 = ps.tile([128, 512], F32, tag="ps", name=f"pT0_{t_id}")
                pT8 = ps.tile([128, 512], F32, tag="ps", name=f"pT8_{t_id}")
                transpose_into(z_sb, pT0, pT8, 0)
                zb = tmpp.tile([128, 512], BF16, tag="zb", name=f"zb{t_id}")
                a_sb = tmpp.tile([128, 512], F32, tag="a", name=f"a{t_id}")
                for hh, pT in ((0, pT0), (1, pT8)):
                    s = slice(256 * hh, 256 * hh + 256)
                    nc.vector.tensor_scalar(zb[:, s], pT[:, 0:256], -1.0, 1.0,
                                            OP.mult, OP.add)
                    nc.vector.tensor_mul(a_sb[:, s], pT[:, 0:256], hT[:, s])

                if fill_a is not None:
                    fill_a()

                # n = tanh(in + r*hn): r*hn on DVE (fp16), injected into the
                # `in` PSUM group via identity matmul duos; tanh + n-transpose
                # pipelined by column halves with filler between PE stages
                rhn = tmpp.tile([128, 512], FP16, tag="rhn", name=f"rhn{t_id}")
                n_sb = gsb.tile([128, 512], BF16, tag="n", name=f"n{t_id}")
                for cc in (0, 1):
                    s = slice(256 * cc, 256 * cc + 256)
                    nc.vector.tensor_mul(rhn[:, s], r_sb[:, s], g["hn"][:, s])
                    nc.tensor.matmul(g["in"][0:64, s], iden16[:, 0:64],
                                     rhn[:, s], start=False, stop=True,
                                     tile_position=(0, 0))
                    nc.tensor.matmul(g["in"][64:128, s], iden16[:, 64:128],
                                     rhn[:, s], start=False, stop=True,
                                     tile_position=(0, 64))

                _f(0)

                for cc in (0, 1):
                    s = slice(256 * cc, 256 * cc + 256)
                    nc.scalar.activation(n_sb[:, s], g["in"][:, s], AF.Tanh)
                    for j in (2 * cc, 2 * cc + 1):
                        nc.tensor.matmul(
                            pT0[:, 256 + 64 * j : 256 + 64 * j + 64],
                            n_sb[0:64, 128 * j : 128 * j + 128],
                            iden2[0:64, :], start=True, stop=True,
                            tile_position=(0, 0))
                        nc.tensor.matmul(
                            pT8[:, 256 + 64 * j : 256 + 64 * j + 64],
                            n_sb[64:128, 128 * j : 128 * j + 128],
                            iden2[64:128, :], start=True, stop=True,
                            tile_position=(64, 0))
                    if cc == 0:
                        _f(1)

                _f(2)

                # hT' = A + zbT*nT by [128,128] quarters; khi01 and khi45
                # depend only on tanh-lo, so they run before khi23/khi67
                # (tanh-hi) in the in-order DVE queue
                hT_new = hst.tile([128, 512], BF16, tag="h", name=f"h{t_id}")
                for q, pT in ((0, pT0), (2, pT8), (1, pT0), (3, pT8)):
                    s = slice(128 * q, 128 * q + 128)
                    ns = slice(256 + 128 * (q % 2), 256 + 128 * (q % 2) + 128)
                    d = tmpp.tile([128, 128], F32, tag="d", name=f"d{t_id}_{q}")
                    nc.vector.tensor_mul(d[:], zb[:, s], pT[:, ns])
                    nc.vector.tensor_add(hT_new[:, s], a_sb[:, s], d[:])
                hT = hT_new

            # ================= encoder =================
            def seed_and_gi_zr(t_id, xt):
                g = {}
                g["z"] = seed_tile("pz", t_id, bias_enc, "z")
                g["r"] = seed_tile("pr", t_id, bias_enc, "r")
                emit_gi(g, xt, ("z",))
                emit_gi(g, xt, ("r",), ks=(0, 1))
                return g

            cur = seed_and_gi_zr(0, xt_tiles[0])
            emit_gi(cur, xt_tiles[0], ("r",), ks=(2, 3))
            cur["in"] = seed_tile("pi", 0, bias_enc, "in")
            emit_gi(cur, xt_tiles[0], ("in",))
            cur["hn"] = seed_tile("ph", 0, bias_enc, "hn")

            for t in range(n_steps):
                if t + 3 < n_steps:
                    xt_tiles[t + 3] = xts.tile([128, 4, BL], BF16, tag="x",
                                               name=f"xt{t+3}")
                    for k in range(4):
                        nc.sync.dma_start(xt_tiles[t + 3][:, k, :], xT_d[t + 3, k])
                emit_gh(cur, whh, ENC_C0, ("z", "r"))
                emit_gh(cur, whh, {"hn": 2 * H}, ("hn",))
                g = cur
                nxt = {}
                if t + 1 < n_steps:
                    xt_next = xt_tiles[t + 1]

                    def fill_a(nxt=nxt, xt_next=xt_next, t=t):
                        nxt.update(seed_and_gi_zr(t + 1, xt_next))

                    def fb1(nxt=nxt, xt_next=xt_next, t=t):
                        nxt["in"] = seed_tile("pi", t + 1, bias_enc, "in")
                        emit_gi(nxt, xt_next, ("in",), ks=(0, 1))

                    def fb2(nxt=nxt, xt_next=xt_next):
                        emit_gi(nxt, xt_next, ("in",), ks=(2, 3))

                    def fb3(nxt=nxt, xt_next=xt_next, t=t):
                        emit_gi(nxt, xt_next, ("r",), ks=(2, 3))
                        nxt["hn"] = seed_tile("ph", t + 1, bias_enc, "hn")

                    fills = (fb1, fb2, fb3)
                else:
                    fill_a, fills = None, ()
                step_tail(t, g, fill_a, fills)
                cur = nxt
                xt_tiles.pop(t, None)

            # ================= decoder =================
            def dec_seeds(g, t_id, gates):
                for gate in gates:
                    g[gate] = seed_tile(f"d{gate}", t_id, bias_dec, gate)
                return g

            def emit_zfill(hA, hB, o0, o1, ks):
                """Column-duo z-output for two steps: top half <- hA (out o0),
                bottom <- hB (out o1), k-chunks `ks` of the accumulation."""
                t = zfill_ps[0]
                for k in ks:
                    dup(t, hA[:, 64 * k : 64 * k + 64], wz[:, k, :],
                        hB[:, 64 * k : 64 * k + 64], wz[:, k, :],
                        start=False, stop=(k == 7))
                if ks and ks[-1] == 7:
                    zo_sb = zop.tile([128, 512], F32, tag="zo", name=f"zo{o1}")
                    nc.scalar.copy(zo_sb[:], t[:])
                    nc.sync.dma_start(z_d[:, o0, :], zo_sb[0:64, :])
                    nc.sync.dma_start(z_d[:, o1, :], zo_sb[64:128, :])

            def zfill_seed(o1):
                t = ps.tile([128, 512], F32, tag="ps", name=f"pzo{o1}")
                dup(t, sstat[:, :], bz_rep[:, :], sstat[:, :], bz_rep[:, :],
                    start=True, stop=False)
                zfill_ps[0] = t

            # z-output pair (o, o+1) reads entry(o+1)=H_o and entry(o+2)=
            # H_{o+1}; its 9 duos are spread over steps o+3 (seed + k0-3) and
            # o+4 (k4-7 + copy) so every decoder step gets ~4.5 filler duos
            assert n_steps == 1 or n_steps % 2 == 0
            zfill_ps = [None]
            entries = {}
            cur = dec_seeds({}, 1000, ("z", "r", "in", "hn"))
            for t in range(n_steps):
                entries[t] = hT
                emit_gh(cur, wcb, DEC_C0, ("z", "r"))
                emit_gh(cur, wcb, DEC_C0, ("hn",))
                emit_gh(cur, wcb, DEC_C0, ("in",), no_stop=("in",))
                g = cur
                nxt = {}
                last = t + 1 >= n_steps

                def fill_a(nxt=nxt, t=t, last=last):
                    if not last:
                        dec_seeds(nxt, 1001 + t, ("z", "r"))
                    if t % 2 == 1 and t >= 3:
                        zfill_seed(t - 3)
                        emit_zfill(entries[t - 2], entries[t - 1],
                                   t - 3, t - 2, (0,))
                    elif t % 2 == 0 and t >= 4:
                        emit_zfill(entries[t - 3], entries[t - 2],
                                   t - 4, t - 3, (4,))

                def fb1(nxt=nxt, t=t, last=last):
                    if not last:
                        dec_seeds(nxt, 1001 + t, ("in", "hn"))

                def fb2(t=t):
                    if t % 2 == 1 and t >= 3:
                        emit_zfill(entries[t - 2], entries[t - 1],
                                   t - 3, t - 2, (1,))
                    elif t % 2 == 0 and t >= 4:
                        emit_zfill(entries[t - 3], entries[t - 2],
                                   t - 4, t - 3, (5,))

                def fb3(t=t):
                    if t % 2 == 1 and t >= 3:
                        emit_zfill(entries[t - 2], entries[t - 1],
                                   t - 3, t - 2, (2, 3))
                    elif t % 2 == 0 and t >= 4:
                        emit_zfill(entries[t - 3], entries[t - 2],
                                   t - 4, t - 3, (6, 7))

                step_tail(1000 + t, g, fill_a, (fb1, fb2, fb3))
                cur = nxt

            if n_steps >= 4:
                emit_zfill(entries[n_steps - 3], entries[n_steps - 2],
                           n_steps - 4, n_steps - 3, list(range(4, 8)))
            if n_steps >= 2:
                zfill_seed(n_steps - 1)
                emit_zfill(entries[n_steps - 1], hT, n_steps - 2, n_steps - 1,
                           list(range(8)))
            elif n_steps == 1:
                zfill_seed(0)
                emit_zfill(hT, hT, 0, 0, list(range(8)))
    return nc


# ---------------------------------------------------------------- host side
def _prep_shared(enc_Wih, enc_Whh, enc_bih, enc_bhh,
                 dec_Wih, dec_Whh, dec_bih, dec_bhh, Wz, bz):
    bf = ml_dtypes.bfloat16
    f32 = np.float32

    def tobf(a):
        return np.ascontiguousarray(np.asarray(a, f32)).astype(bf)

    wihT = tobf(enc_Wih.T.reshape(I, 3 * H)).reshape(4, 128, 3 * H)
    whhT = tobf(enc_Whh.T).reshape(8, 128, 3 * H)
    wcomb = np.concatenate(
        [dec_Wih[: 2 * H] + dec_Whh[: 2 * H], dec_Wih[2 * H :], dec_Whh[2 * H :]], 0
    )
    wcombT = tobf(wcomb.T).reshape(8, 128, 4 * H)
    wzT = tobf(np.asarray(Wz, f32).T).reshape(8, 128, I)

    def rep(row):
        return np.broadcast_to(np.asarray(row, f32)[None, :], (128, row.shape[0]))

    be = np.concatenate([np.asarray(enc_bih, f32)[: 2 * H]
                         + np.asarray(enc_bhh, f32)[: 2 * H],
                         np.asarray(enc_bih, f32)[2 * H :],
                         np.asarray(enc_bhh, f32)[2 * H :]])
    bd = np.concatenate([np.asarray(dec_bih, f32)[: 2 * H]
                         + np.asarray(dec_bhh, f32)[: 2 * H],
                         np.asarray(dec_bih, f32)[2 * H :],
                         np.asarray(dec_bhh, f32)[2 * H :]])
    iden2 = np.concatenate([np.eye(64, dtype=f32)] * 2, axis=0)
    iden16 = np.zeros((128, 128), f32)
    iden16[0:64, 0:64] = np.eye(64, dtype=f32)
    iden16[64:128, 64:128] = np.eye(64, dtype=f32)
    return {
        "wihT": wihT, "whhT": whhT, "wcombT": wcombT, "wzT": wzT,
        "bias_enc": tobf(rep(be)), "bias_dec": tobf(rep(bd)),
        "bz_rep": tobf(rep(np.asarray(bz, f32))),
        "sstat": np.full((128, 64), 1.0 / 128, f32).astype(bf),
        "iden2": tobf(iden2),
        "iden16": iden16.astype(np.float16),
        "h0T": np.full((128, 512), 0.1, f32).astype(bf),
    }


def kernel(x, enc_Wih, enc_Whh, enc_bih, enc_bhh,
           dec_Wih, dec_Whh, dec_bih, dec_bhh, Wz, bz, n_steps=T):
    x = np.asarray(x, np.float32)
    shared = _prep_shared(enc_Wih, enc_Whh, enc_bih, enc_bhh,
                          dec_Wih, dec_Whh, dec_bih, dec_bhh, Wz, bz)
    in_maps = []
    for c in range(N_CORES):
        xc = x[c * BL : (c + 1) * BL, :n_steps]  # [BL, n_steps, I]
        xT = np.ascontiguousarray(xc.transpose(1, 2, 0)).reshape(n_steps, 4, 128, BL)
        in_maps.append({"xT": xT.astype(ml_dtypes.bfloat16), **shared})

    nc = build_nc(n_steps)
    _split_multi_waits(nc)

    trace = bool(int(os.environ.get("GRU_TRACE", "0")))
    if trace:
        _install_ntff_hook()
    res = bass_utils.run_bass_kernel_spmd(
        nc, in_maps, core_ids=list(range(N_CORES)), trace=trace
    )
    if trace and res.exec_time_ns is not None:
        print(f"HW exec time: {res.exec_time_ns} ns")
    out = np.concatenate([res.results[c]["z"] for c in range(N_CORES)], axis=0)
    return out
